# revision 29
# baseline (speedup 1.0000x reference)
"""Bass/Trainium2 kernel for a single LSTM-cell step + tiny MLP head.

Reference computation (all fp32):
    gates = W_ih @ x + b_ih + W_hh @ h0 + b_hh        # [4H], gate order i,f,g,o
    i, f, g, o = sigmoid/sigmoid/tanh/sigmoid splits
    c = f * c0 + i * g ; h = o * tanh(c)              # [H]
    z = relu(W1 @ h + b1)                             # [32]
    out = sigmoid(W2 @ z + b2)                        # [130]

Sharding (8 NeuronCores, tensor-parallel over the hidden dim): core k owns
hidden slice s_k = [k*512, (k+1)*512) of every gate; the big matvec streams
the core's W_ih row-block through TensorE (weights moving, x stationary)
accumulating into PSUM; a [32]-float AllReduce combines the per-core partial
MLP dots and every core finishes the replicated head.

Fast path (h0 == 0 and c0 == 0, which this model's inputs guarantee):
  * the W_hh stream is dropped entirely (W_hh @ 0 == 0, exact), and
  * the f-gate rows of W_ih are dropped (c = f*c0 + i*g == i*g, exact), so
    only the i/g/o row-blocks stream: R = 3*512 rows per core, and
  * weights/x are stored fp8e4m3 (W scaled by 16, x by 1/16; measured end-to-
    end max rel err ~1e-3 vs the 2e-2 budget), with DoubleRow matmuls (2
    K-tiles per instruction) so the PE stays well ahead of the DMA stream.
  Per-core HBM traffic: 65 K-tiles * 1536 rows * 128 B = 12.8 MB.

General fallback (any nonzero h0/c0): all four gates + the W_hh stream, all
bf16 unscaled — compiled only if actually needed.

Epilogue runs partition-parallel: the activated gate row [1, R] is
transposed on the PE (K=1 matmuls against a ones vector) into [128, R/128],
so the c/h elementwise math uses 128 lanes and h lands directly in the
[128, 4] layout the z = W1_k @ h_k matmul needs (no DRAM round-trip).
b1/8 is folded into each core's z partial pre-AllReduce; b2 is folded into
the head matmul as an extra constant-one row.

A single dependency-free dummy AllReduce issued at kernel start pays the CC
bootstrap (~50us) underneath the weight stream so the real [32]-float
AllReduce runs warm. Dummy matmuls on resident SBUF data pad each DMA
group's PE work so the PE never idles back to half clock (HAM).
"""

import os

import numpy as np
import ml_dtypes

D = 8196
H = 4096
HS = 512           # hidden slice per core
HID = 32
OUT = 130
NCORES = 8
MMN = 512          # matmul free dim = one PSUM bank

K1D = D + 1        # x ++ 1.0 (bias column)
K1T = 65           # ceil(8197/128) K-tiles
K1P = K1T * 128
K2T = H // 128     # 32 (general path only)
SC = 16.0          # fp8 weight scale (x scaled by 1/SC)

G = int(os.environ.get("KERNEL_G", "8"))      # K-tiles per weight DMA group
WBUFS = int(os.environ.get("KERNEL_BUFS", "4"))
BW = float(os.environ.get("KERNEL_BW", "300"))  # est. stream B/ns for PE padding
# 16 so the DoubleRow stationary AP's K-pair stride is 16 B (ISA minimum)
MREP = int(os.environ.get("KERNEL_MREP", "16"))
DR = os.environ.get("KERNEL_DR", "1") == "1"        # fp8 DoubleRow matmuls
# scalar HWDGE sustains only ~60 B/ns for bulk (vs sync ~300) — keep off
QSPLIT = os.environ.get("KERNEL_QSPLIT", "0") == "1"
NDUMAR = int(os.environ.get("KERNEL_NDUMAR", "1"))  # warm-up AllReduces
DUMMY = os.environ.get("KERNEL_DUMMY", "auto")      # PE-warm dummies per group
HOTDT = os.environ.get("KERNEL_WDT", "fp8")         # fast-path weight dtype
STAGE = os.environ.get("KERNEL_STAGE", "full")      # debug: "z" stops pre-CC
CCKIND = os.environ.get("KERNEL_CC", "ar")          # "ar" AllReduce | "ag" AllGather
ZRT = os.environ.get("KERNEL_ZRT", "1") == "1"      # reload z via PE transpose

_WDTS = {
    "bf16": ml_dtypes.bfloat16,
    "fp8": ml_dtypes.float8_e4m3fn,
    "fp32": np.float32,
}

_cached = {}


def _mybir_dt(mybir, np_dt):
    name = np.dtype(np_dt).name
    return {
        "bfloat16": mybir.dt.bfloat16,
        "float32": mybir.dt.float32,
        "float8_e4m3fn": mybir.dt.float8e4,
    }[name]


def _groups(n_ktiles):
    """DMA group sizes with a small ramp so the PE starts early."""
    sizes = []
    for s in (1, 1, 2):
        if sum(sizes) + s <= n_ktiles:
            sizes.append(s)
    rem = n_ktiles - sum(sizes)
    sizes += [G] * (rem // G)
    if rem % G:
        sizes.append(rem % G)
    return sizes


def build_nc(general):
    """Build + compile the per-core Bass program (same program on all cores)."""
    import concourse.bass as bass
    import concourse.tile as tile
    from concourse import bacc, mybir

    fp32 = mybir.dt.float32
    AF = mybir.ActivationFunctionType

    if general:
        w_np = _WDTS["bf16"]
        ng = 4
    else:
        w_np = _WDTS[HOTDT]
        ng = 3
    dt1 = _mybir_dt(mybir, w_np)
    use_dr = DR and dt1 == mybir.dt.float8e4
    R = ng * HS
    NB = R // MMN
    NT = R // 128           # transposed-gate columns
    esz = {mybir.dt.bfloat16: 2, mybir.dt.float32: 4, mybir.dt.float8e4: 1}

    nc = bacc.Bacc("TRN2", target_bir_lowering=False, debug=False,
                   num_devices=NCORES)

    wt1_d = nc.dram_tensor("wt1", [128, K1T * R], dt1, kind="ExternalInput")
    xt1_d = nc.dram_tensor("xt1", [128, K1T * MREP], dt1, kind="ExternalInput")
    if general:
        wt2_d = nc.dram_tensor("wt2", [128, K2T * R], dt1, kind="ExternalInput")
        xt2_d = nc.dram_tensor("xt2", [128, K2T * MREP], dt1,
                               kind="ExternalInput")
        c0_d = nc.dram_tensor("c0t", [128, HS // 128], fp32,
                              kind="ExternalInput")
    w1_d = nc.dram_tensor("w1t", [128, (HS // 128) * HID], fp32,
                          kind="ExternalInput")
    b1_d = nc.dram_tensor("b1s", [HID], fp32, kind="ExternalInput")
    w2_d = nc.dram_tensor("w2a", [HID + 1, OUT], fp32, kind="ExternalInput")
    out_d = nc.dram_tensor("out", [OUT], fp32, kind="ExternalOutput")

    zp_d = nc.dram_tensor("zpart", [HID], fp32)
    if CCKIND == "ag":
        zr_d = nc.dram_tensor("zred", [NCORES * HID], fp32, addr_space="Shared")
    else:
        zr_d = nc.dram_tensor("zred", [HID], fp32, addr_space="Shared")
    # warm-up collective operand: Internal DRAM scratch, never written —
    # its (garbage) contents don't matter and reading it keeps the CC
    # trigger dependency-free
    dum_d = nc.dram_tensor("ccdummy", [HID], fp32)
    dumr_d = nc.dram_tensor(
        "ccdummyr", [NCORES * HID if CCKIND == "ag" else HID], fp32)

    KT = K1T + (K2T if general else 0)

    with tile.TileContext(nc) as tc:
        with (
            tc.tile_pool(name="weights", bufs=WBUFS) as wpool,
            tc.tile_pool(name="small", bufs=1) as small,
            tc.tile_pool(name="psum", bufs=1, space="PSUM") as psum,
        ):
            # Optional dependency-free warm-up AllReduce. Measured: the CC
            # bootstrap ("Invalid", ~21+45us) runs autonomously, and the
            # first 8-core op after it costs ~11us regardless — so the
            # fastest chain is the real AllReduce as the first CC op
            # (NDUMAR=0). A pairwise dummy actively hurts (ring reconfig
            # makes the following 8-core op ~19.5us).
            for _ in range(NDUMAR):
                nc.gpsimd.collective_compute(
                    "AllGather" if CCKIND == "ag" else "AllReduce",
                    mybir.AluOpType.bypass if CCKIND == "ag"
                    else mybir.AluOpType.add,
                    replica_groups=[list(range(NCORES))],
                    ins=[dum_d[:]],
                    outs=[dumr_d[:]],
                )

            # small persistent operands on the ACT HWDGE ring
            xt1_sb = small.tile([128, K1T * MREP], dt1)
            nc.scalar.dma_start(xt1_sb[:], xt1_d[:])
            if general:
                xt2_sb = small.tile([128, K2T * MREP], dt1)
                nc.scalar.dma_start(xt2_sb[:], xt2_d[:])
                c0_sb = small.tile([128, HS // 128], fp32)
                nc.scalar.dma_start(c0_sb[:], c0_d[:])
            w1_sb = small.tile([128, HS // 128, HID], fp32)
            nc.scalar.dma_start(w1_sb[:], w1_d[:])
            b1_sb = small.tile([1, HID], fp32)
            nc.scalar.dma_start(b1_sb[:], b1_d[None, :])
            w2_sb = small.tile([HID + 1, OUT], fp32)
            nc.scalar.dma_start(w2_sb[:], w2_d[:])

            ones1 = small.tile([1, 1], fp32)
            nc.gpsimd.memset(ones1[:], 1.0)
            # head operand [33, 1]: rows 0..31 get relu(z) later, row 32
            # stays 1.0 so the matmul picks up the b2 row of w2a
            zaug = small.tile([HID + 1, 1], fp32)
            nc.gpsimd.memset(zaug[:], 1.0)

            # resident garbage operand + scratch PSUM bank for PE-warming
            # dummy matmuls
            dmy_sb = small.tile([128, MMN], dt1)
            nc.gpsimd.memset(dmy_sb[:], 0.0)
            dmy_ps = psum.tile([MREP, MMN], fp32)

            gates_ps = psum.tile([MREP, R], fp32)

            xt1_3 = xt1_sb[:].rearrange("p (t m) -> p t m", m=MREP)
            segs = [(wt1_d, xt1_3, _groups(K1T))]
            if general:
                xt2_3 = xt2_sb[:].rearrange("p (t m) -> p t m", m=MREP)
                segs = [(wt2_d, xt2_3, _groups(K2T))] + segs

            kk = 0          # global K-tile index for start/stop flags
            gi = 0          # DMA group index for ring alternation
            for wt_d, xt_3, group_sizes in segs:
                g0 = 0
                for gs in group_sizes:
                    wtile = wpool.tile([128, G * R], dt1, tag="wtile")
                    eng = nc.scalar if (QSPLIT and gi % 2) else nc.sync
                    eng.dma_start(wtile[:, : gs * R],
                                  wt_d[:, g0 * R:(g0 + gs) * R])
                    gi += 1
                    w3 = wtile[:, : gs * R].rearrange("p (t r) -> p t r", r=R)
                    t = 0
                    nmm = 0
                    while t < gs:
                        pair = 2 if (use_dr and t + 1 < gs) else 1
                        for nb in range(NB):
                            if pair == 2:
                                nc.tensor.matmul(
                                    gates_ps[:, nb * MMN:(nb + 1) * MMN],
                                    lhsT=xt_3[:, g0 + t:g0 + t + 2, :],
                                    rhs=w3[:, t:t + 2, nb * MMN:(nb + 1) * MMN],
                                    start=(kk == 0),
                                    stop=(kk + 2 == KT),
                                    perf_mode=mybir.MatmulPerfMode.DoubleRow,
                                )
                            else:
                                nc.tensor.matmul(
                                    gates_ps[:, nb * MMN:(nb + 1) * MMN],
                                    lhsT=xt_3[:, g0 + t, :],
                                    rhs=w3[:, t, nb * MMN:(nb + 1) * MMN],
                                    start=(kk == 0),
                                    stop=(kk + 1 == KT),
                                )
                            nmm += 1
                        t += pair
                        kk += pair
                    # pad PE work up to the group's DMA time so the PE never
                    # idles (idle gaps drop it to half clock)
                    if DUMMY == "auto":
                        dma_ns = 128 * gs * R * esz[dt1] / BW
                        pe_ns = nmm * 221
                        ndum = max(0, int(1.2 * (dma_ns - pe_ns) / 230))
                    else:
                        ndum = int(DUMMY)
                    for _ in range(ndum):
                        nc.tensor.matmul(dmy_ps[:], lhsT=dmy_sb[:, 0:MREP],
                                         rhs=dmy_sb[:], start=True, stop=True)
                    g0 += gs

            # ---- epilogue ----
            # activate the gate row: [1, R] on PSUM partition 0. Row blocks
            # are ordered [i | o | g] (general: [i | f | o | g]) so the
            # sigmoid gates form one contiguous slice -> 2 activation ops.
            gsb = small.tile([1, R], fp32)
            nsig = (ng - 1) * HS
            nc.scalar.activation(gsb[:, :nsig], gates_ps[0:1, :nsig],
                                 AF.Sigmoid)
            nc.scalar.activation(gsb[:, nsig:], gates_ps[0:1, nsig:R],
                                 AF.Tanh)

            # PE-transpose the activated gates [1, R] -> [128, NT] so the
            # c/h math runs on 128 lanes and h lands matmul-ready
            tp_ps = psum.tile([128, NT], fp32)
            for c in range(NT):
                nc.tensor.matmul(tp_ps[:, c:c + 1],
                                 lhsT=gsb[:, c * 128:(c + 1) * 128],
                                 rhs=ones1[:], start=True, stop=True)
            tsb = small.tile([128, NT], fp32)
            nc.vector.tensor_copy(tsb[:], tp_ps[:])

            # transposed gate columns ordered [i | o | g] / [i | f | o | g]
            nq = HS // 128  # 4 columns per gate
            g_sl = tsb[:, (ng - 1) * nq:ng * nq]
            o_sl = tsb[:, (ng - 2) * nq:(ng - 1) * nq]
            c_sb = small.tile([128, nq], fp32)
            if general:
                ig = small.tile([128, nq], fp32)
                nc.vector.tensor_mul(ig[:], tsb[:, 0:nq], g_sl)
                fc = small.tile([128, nq], fp32)
                nc.vector.tensor_mul(fc[:], tsb[:, nq:2 * nq], c0_sb[:])
                nc.vector.tensor_add(c_sb[:], fc[:], ig[:])
            else:
                nc.vector.tensor_mul(c_sb[:], tsb[:, 0:nq], g_sl)
            th_sb = small.tile([128, nq], fp32)
            nc.scalar.activation(th_sb[:], c_sb[:], AF.Tanh)
            h_sb = small.tile([128, nq], fp32)
            nc.vector.tensor_mul(h_sb[:], o_sl, th_sb[:])

            # partial MLP layer 1: z_part = W1[:, s_k] @ h_k + b1/8 -> [32]
            z_ps = psum.tile([1, HID], fp32)
            for t in range(nq):
                nc.tensor.matmul(z_ps[:], lhsT=h_sb[:, t:t + 1],
                                 rhs=w1_sb[:, t, :], start=(t == 0), stop=False)
            nc.tensor.matmul(z_ps[:], lhsT=ones1[:], rhs=b1_sb[:],
                             start=False, stop=True)

            z_sb = small.tile([1, HID], fp32)
            nc.vector.tensor_copy(z_sb[:], z_ps[0:1, :])
            if STAGE == "z":
                nc.scalar.dma_start(out_d[None, :HID], z_sb[:])
            else:
                nc.scalar.dma_start(zp_d[None, :], z_sb[:])
                if CCKIND == "ag":
                    nc.gpsimd.collective_compute(
                        "AllGather",
                        mybir.AluOpType.bypass,
                        replica_groups=[list(range(NCORES))],
                        ins=[zp_d[:]],
                        outs=[zr_d[:]],
                    )
                    # reload gathered z as [32, 8] and reduce over cores
                    zg_sb = small.tile([HID, NCORES], fp32)
                    nc.scalar.dma_start(
                        zg_sb[:], zr_d.ap().rearrange("(c e) -> e c", e=HID))
                    zr_sb = small.tile([HID, 1], fp32)
                    nc.vector.tensor_reduce(zr_sb[:], zg_sb[:],
                                            mybir.AxisListType.X,
                                            mybir.AluOpType.add)
                else:
                    nc.gpsimd.collective_compute(
                        "AllReduce",
                        mybir.AluOpType.add,
                        replica_groups=[list(range(NCORES))],
                        ins=[zp_d[:]],
                        outs=[zr_d[:]],
                    )
                    if ZRT and not general:  # general path is out of PSUM banks
                        # contiguous [1,32] reload (1 descriptor) + PE
                        # transpose beats the 32-partition scatter DMA
                        zrow = small.tile([1, HID], fp32)
                        nc.scalar.dma_start(zrow[:], zr_d[None, :])
                        zt_ps = psum.tile([HID, 1], fp32)
                        nc.tensor.matmul(zt_ps[:], lhsT=zrow[:],
                                         rhs=ones1[:], start=True, stop=True)
                        zr_sb = zt_ps
                    else:
                        # reload reduced z as [32,1] (partition-per-element)
                        zr_sb = small.tile([HID, 1], fp32)
                        nc.scalar.dma_start(zr_sb[:], zr_d[:, None])
                nc.scalar.activation(zaug[0:HID, :], zr_sb[:], AF.Relu)

                out_ps = psum.tile([1, OUT], fp32)
                nc.tensor.matmul(out_ps[:], lhsT=zaug[:], rhs=w2_sb[:],
                                 start=True, stop=True)
                res = small.tile([1, OUT], fp32)
                nc.scalar.activation(res[:], out_ps[0:1, :], AF.Sigmoid)
                nc.scalar.dma_start(out_d[None, :], res[:])

    nc.compile()
    return nc


def get_nc(general):
    key = ("gen" if general else "hot")
    if key not in _cached:
        _cached[key] = build_nc(general)
    return _cached[key]


def _ktile(mat, n_kt):
    """[rows(R), K] -> [128, n_kt*R] with out[p, t*R + j] = mat[j, t*128 + p]."""
    r = mat.shape[0]
    return (mat.T.reshape(n_kt, 128, r).transpose(1, 0, 2)
            .reshape(128, n_kt * r))


def shard_inputs(inputs, general):
    """Slice/transpose/cast the full inputs into per-core input maps."""
    x = np.asarray(inputs["x"], np.float32)
    h0 = np.asarray(inputs["h0"], np.float32)
    c0 = np.asarray(inputs["c0"], np.float32)
    W_ih = np.asarray(inputs["W_ih"], np.float32)
    W_hh = np.asarray(inputs["W_hh"], np.float32)
    b = np.asarray(inputs["b_ih"], np.float32) + np.asarray(inputs["b_hh"], np.float32)
    W1 = np.asarray(inputs["W1"], np.float32)
    b1 = np.asarray(inputs["b1"], np.float32)
    W2 = np.asarray(inputs["W2"], np.float32)
    b2 = np.asarray(inputs["b2"], np.float32)

    # gate-block order [i | o | g] (general: [i | f | o | g]) keeps the
    # sigmoid gates contiguous; W_ih stores gates as [i, f, g, o]
    if general:
        w_np = _WDTS["bf16"]
        gsel = (0, 1, 3, 2)
        sc = 1.0
    else:
        w_np = _WDTS[HOTDT]
        gsel = (0, 3, 2)
        sc = SC if w_np == ml_dtypes.float8_e4m3fn else 1.0
    R = len(gsel) * HS

    xc1 = np.zeros(K1P, np.float32)
    xc1[:D] = x / sc
    xc1[D] = 1.0 / sc
    xt1 = np.repeat(xc1.reshape(K1T, 128).T, MREP, axis=1).astype(w_np)
    xt1 = np.ascontiguousarray(xt1)
    if general:
        xt2 = np.repeat(h0.reshape(K2T, 128).T, MREP, axis=1).astype(w_np)
        xt2 = np.ascontiguousarray(xt2)

    w2a = np.ascontiguousarray(np.vstack([W2.T, b2[None, :]]).astype(np.float32))
    b1s = np.ascontiguousarray(b1 / NCORES)

    in_maps = []
    for k in range(NCORES):
        rows = np.concatenate([np.arange(g * H + k * HS, g * H + (k + 1) * HS)
                               for g in gsel])
        Wf1 = np.zeros((R, K1P), np.float32)
        Wf1[:, :D] = W_ih[rows] * sc
        Wf1[:, D] = b[rows] * sc
        wt1 = _ktile(Wf1, K1T).astype(w_np)
        # W1 slice, transposed and K-tiled:
        #   w1t[p, t*HID + j] = W1[j, k*HS + t*128 + p]
        w1t = (W1[:, k * HS:(k + 1) * HS].T
               .reshape(HS // 128, 128, HID).transpose(1, 0, 2)
               .reshape(128, (HS // 128) * HID))
        m = {
            "wt1": wt1,
            "xt1": xt1,
            "w1t": np.ascontiguousarray(w1t),
            "b1s": b1s,
            "w2a": w2a,
        }
        if general:
            m["wt2"] = _ktile(W_hh[rows], K2T).astype(w_np)
            m["xt2"] = xt2
            m["c0t"] = np.ascontiguousarray(
                c0[k * HS:(k + 1) * HS].reshape(HS // 128, 128).T)
        in_maps.append(m)
    return in_maps


def run(inputs, trace=False, general=None):
    from concourse.bass_utils import run_bass_kernel_spmd
    if general is None:
        general = bool(np.any(np.asarray(inputs["h0"]))) or \
            bool(np.any(np.asarray(inputs["c0"])))
    nc = get_nc(general)
    in_maps = shard_inputs(inputs, general)
    return run_bass_kernel_spmd(nc, in_maps, list(range(NCORES)), trace=trace)


def kernel(**inputs) -> np.ndarray:
    res = run(inputs, trace=False)
    return np.asarray(res.results[0]["out"], np.float32)
